# revision 1
# baseline (speedup 1.0000x reference)
"""CrossAttentionFusion Trainium2 kernel.

Full inputs -> shard (batch x query-half) over 8 NeuronCores -> full output.

Per core (batch b = core//2, query half h = core%2, NH=2048 queries):
  Algebraic folding (host precompute):
    L[m,n] = K^T Q = x2^T (k_w^T q_w) x1 =: x2^T Q'   (K never materialized;
             terms constant in m cancel in softmax; x2^T k_w^T q_b folds
             into Q' channel bias)
    F_att   = v_w (x2 A_norm) + v_b  ->  M1 = (proj_w v_w) Z,  Z = x2 E
             (V never materialized; proj_w v_w and proj_w v_b precomputed)
  Device per 512-query block:
    L[m, n] = x2^T Q'                (fp32r matmuls, m on partitions)
    E = exp(L / 16)                  (ACT; no max subtraction: logits O(1))
    S[n] = sum_m E[m, n]             (DVE running sum + one PE reduce)
    Z[c, n] = sum_m x2[c, m] E[m, n] (lhsT = host-pretransposed x2)
    M1 = P2 Z ;  out = x1 + gate * relu(M1 * G * (1/S) + Bc)
  with G = gamma*rsqrt(var+eps), Bc = beta + (proj_b + proj_w v_b - mean)*G.
  fusion(j-1) is interleaved into logits(j) on the PE; exp and the softmax
  sum run on ACT/DVE one step behind; 1/S is hidden under the next block.

Everything on the PE runs in float32r (~2e-4 matmul rel err, full rate).
"""
from contextlib import ExitStack

import numpy as np

import concourse.bass as bass
import concourse.mybir as mybir
import concourse.tile as tile
from concourse import bacc
from concourse.bass_utils import run_bass_kernel_spmd

F32 = mybir.dt.float32
F32R = mybir.dt.float32r
AF = mybir.ActivationFunctionType
OP = mybir.AluOpType

B, C, H, W = 4, 256, 64, 64
N = H * W            # 4096
NCORES = 8
NH = N // 2          # 2048 queries per core
NBLK = 512           # query block
NBLOCKS = NH // NBLK
MT = N // 128        # 32 m-tiles
EPS = 1e-5
SCALE = float(C) ** -0.5


def build():
    nc = bacc.Bacc("TRN2", target_bir_lowering=False, debug=False,
                   num_devices=NCORES)
    x1r_d = nc.dram_tensor("x1r", [C, NH], F32R, kind="ExternalInput")
    x2r_d = nc.dram_tensor("x2r", [C, N], F32R, kind="ExternalInput")
    x2t_d = nc.dram_tensor("x2t", [128, MT * C], F32R, kind="ExternalInput")
    wm_d = nc.dram_tensor("wmat", [C, 2 * C], F32R, kind="ExternalInput")
    gw_d = nc.dram_tensor("gw", [C, 2], F32R, kind="ExternalInput")
    vec_d = nc.dram_tensor("vecs", [C, 4], F32, kind="ExternalInput")
    gb_d = nc.dram_tensor("gateb", [1, 1], F32, kind="ExternalInput")
    out_d = nc.dram_tensor("out", [C, NH], F32, kind="ExternalOutput")

    with tile.TileContext(nc) as tc, ExitStack() as ctx:
        pers = ctx.enter_context(tc.tile_pool(name="pers", bufs=1))
        work = ctx.enter_context(tc.tile_pool(name="work", bufs=2))
        psum = ctx.enter_context(tc.tile_pool(name="psum", bufs=1, space="PSUM"))

        # ---- persistent tiles ----
        wm = [pers.tile([128, 2 * C], F32R, tag=f"wm{ci}", name=f"wm{ci}") for ci in range(2)]
        gw = [pers.tile([128, 2], F32R, tag=f"gw{ci}", name=f"gw{ci}") for ci in range(2)]
        vec = [pers.tile([128, 4], F32, tag=f"vec{ci}", name=f"vec{ci}") for ci in range(2)]
        gb = pers.tile([1, 1], F32, tag="gb", name="gb")
        x2r = [pers.tile([128, N], F32R, tag=f"x2r{ci}", name=f"x2r{ci}") for ci in range(2)]
        x2t = pers.tile([128, MT * C], F32R, tag="x2t", name="x2t")
        Qt = [pers.tile([128, NH], F32R, tag=f"Qt{co}", name=f"Qt{co}") for co in range(2)]
        grow = pers.tile([1, NH], F32R, tag="grow", name="grow")
        ones_f = pers.tile([128, 1], F32, tag="ones_f", name="ones_f")
        ones_f2 = pers.tile([1, 128], F32, tag="ones_f2", name="ones_f2")
        ones_c = pers.tile([128, 1], F32R, tag="ones_c", name="ones_c")
        ones_k1 = pers.tile([1, 128], F32R, tag="ones_k1", name="ones_k1")

        # E pool created before xin so both coexist (budgeted); xin's
        # release after gate frees its space for good.
        epool = ctx.enter_context(tc.tile_pool(name="epool", bufs=1))
        E = epool.tile([128, MT * NBLK], F32R, tag="E", name="E")

        def fusion_mms(fp, mt):
            es = slice(mt * NBLK, (mt + 1) * NBLK)
            for co in range(2):
                nc.tensor.matmul(
                    fp[co][:], x2t[:, mt * C + co * 128: mt * C + (co + 1) * 128],
                    E[:, es], start=(mt == 0), stop=(mt == MT - 1))

        def sacc_adds(sacc, mt2):
            e0 = slice((2 * mt2) * NBLK, (2 * mt2 + 1) * NBLK)
            e1 = slice((2 * mt2 + 1) * NBLK, (2 * mt2 + 2) * NBLK)
            if mt2 == 0:
                nc.vector.tensor_add(sacc[:], E[:, e0], E[:, e1])
            else:
                nc.vector.tensor_add(sacc[:], sacc[:], E[:, e0])
                nc.vector.tensor_add(sacc[:], sacc[:], E[:, e1])

        def s_finalize(j, sacc):
            with nc.named_scope(f"sfin{j}"):
                sp = psum.tile([1, NBLK], F32, tag="s", name="s", bufs=1)
                nc.tensor.matmul(sp[:], ones_c[:], sacc[:])
                invs_f = work.tile([1, NBLK], F32, tag="invs_f", name="invs_f",
                                   bufs=1)
                nc.vector.reciprocal_approx_fast(invs_f[:], sp[:])
                invs_r = work.tile([1, NBLK], F32R, tag="invs_r", name="invs_r",
                                   bufs=1)
                nc.vector.tensor_copy(invs_r[:], invs_f[:])
            return invs_r

        def post_block(j, fp, invs_r):
            ns = slice(j * NBLK, (j + 1) * NBLK)
            with nc.named_scope(f"post{j}"):
                Fs = [work.tile([128, NBLK], F32R, tag=f"Fs{co}", name=f"Fs{co}",
                                bufs=1) for co in range(2)]
                for co in range(2):
                    nc.scalar.activation(Fs[co][:], fp[co][:], AF.Copy)
                bc1 = psum.tile([128, NBLK], F32, tag="acc", name="acc", bufs=3)
                nc.tensor.matmul(bc1[:], ones_k1[:], invs_r[:])
                invs_b = work.tile([128, NBLK], F32, tag="invs_b", name="invs_b",
                                   bufs=1)
                nc.vector.tensor_copy(invs_b[:], bc1[:])
                bc2 = psum.tile([128, NBLK], F32, tag="acc", name="acc", bufs=3)
                nc.tensor.matmul(bc2[:], ones_k1[:], grow[:, ns])
                gate_b = work.tile([128, NBLK], F32, tag="gate_b", name="gate_b",
                                   bufs=1)
                nc.vector.tensor_copy(gate_b[:], bc2[:])
                for co in range(2):
                    cs = slice(co * 128, (co + 1) * 128)
                    mp = psum.tile([128, NBLK], F32, tag="acc", name="acc", bufs=3)
                    for ci in range(2):
                        nc.tensor.matmul(
                            mp[:], wm[ci][:, C + co * 128: C + (co + 1) * 128],
                            Fs[ci][:], start=(ci == 0), stop=(ci == 1))
                    x1t = work.tile([128, NBLK], F32R, tag="x1t", name="x1t")
                    nc.sync.dma_start(x1t[:], x1r_d[cs, ns])
                    t1 = work.tile([128, NBLK], F32, tag="t1", name="t1")
                    nc.vector.scalar_tensor_tensor(
                        t1[:], mp[:], vec[co][:, 1:2], invs_b[:],
                        op0=OP.mult, op1=OP.mult)
                    r = work.tile([128, NBLK], F32, tag="r", name="r")
                    nc.scalar.activation(r[:], t1[:], AF.Relu,
                                         bias=vec[co][:, 2:3])
                    rg = work.tile([128, NBLK], F32, tag="t1", name="rg")
                    nc.gpsimd.tensor_mul(rg[:], r[:], gate_b[:])
                    ot = work.tile([128, NBLK], F32, tag="ot", name="ot")
                    nc.gpsimd.tensor_add(ot[:], rg[:], x1t[:].bitcast(F32))
                    nc.sync.dma_start(out_d[cs, ns], ot[:])

        def emit_block(blk, prev_fp, sacc):
            ns = slice(blk * NBLK, (blk + 1) * NBLK)
            for mt2 in range(MT // 2):
                lp = psum.tile([128, 2 * NBLK], F32, tag="L", name="L", bufs=2)
                for sub in range(2):
                    mt = 2 * mt2 + sub
                    msl = slice(mt * 128, (mt + 1) * 128)
                    for ci in range(2):
                        nc.tensor.matmul(
                            lp[:, sub * NBLK:(sub + 1) * NBLK],
                            x2r[ci][:, msl], Qt[ci][:, ns],
                            start=(ci == 0), stop=(ci == 1))
                if prev_fp is not None:
                    fusion_mms(prev_fp, 2 * mt2)
                    fusion_mms(prev_fp, 2 * mt2 + 1)
                nc.scalar.activation(
                    E[:, mt2 * 2 * NBLK:(mt2 + 1) * 2 * NBLK], lp[:],
                    AF.Exp, scale=SCALE)
                if mt2 > 0:
                    sacc_adds(sacc, mt2 - 1)
            sacc_adds(sacc, MT // 2 - 1)

        with nc.named_scope("pre"):
            nc.sync.dma_start(wm[0][:], wm_d[0:128, :])
            nc.gpsimd.dma_start(wm[1][:], wm_d[128:256, :])
            nc.vector.memset(ones_f[:], 1.0)
            nc.vector.tensor_copy(ones_c[:], ones_f[:])
            nc.vector.memset(ones_f2[:], 1.0)
            nc.vector.tensor_copy(ones_k1[:], ones_f2[:])

        sacc0 = None
        with tc.tile_pool(name="xin", bufs=1) as xin:
            x1r = [xin.tile([128, NH], F32R, tag=f"x1r{ci}", name=f"x1r{ci}") for ci in range(2)]
            with nc.named_scope("pre"):
                CH = 1024
                # interleave x1/x2 chunks: Q' and logits0 stream against arrivals
                nc.sync.dma_start(x1r[0][:, 0:CH], x1r_d[0:128, 0:CH])
                nc.gpsimd.dma_start(x1r[1][:, 0:CH], x1r_d[128:256, 0:CH])
                nc.sync.dma_start(x2r[0][:, 0:CH], x2r_d[0:128, 0:CH])
                nc.gpsimd.dma_start(x2r[1][:, 0:CH], x2r_d[128:256, 0:CH])
                nc.sync.dma_start(x1r[0][:, CH:NH], x1r_d[0:128, CH:NH])
                nc.gpsimd.dma_start(x1r[1][:, CH:NH], x1r_d[128:256, CH:NH])
                for ch in range(1, N // CH):
                    chs = slice(ch * CH, (ch + 1) * CH)
                    nc.sync.dma_start(x2r[0][:, chs], x2r_d[0:128, chs])
                    nc.gpsimd.dma_start(x2r[1][:, chs], x2r_d[128:256, chs])
                for ci in range(2):
                    cs = slice(ci * 128, (ci + 1) * 128)
                    nc.sync.dma_start(gw[ci][:], gw_d[cs, :])
                    nc.sync.dma_start(vec[ci][:], vec_d[cs, :])
                nc.sync.dma_start(gb[:], gb_d[:])
                nc.sync.dma_start(x2t[:, 0: MT * C // 2], x2t_d[:, 0: MT * C // 2])
                nc.gpsimd.dma_start(x2t[:, MT * C // 2:], x2t_d[:, MT * C // 2:])

                # Q' projection
                for co in range(2):
                    for nch in range(NH // NBLK):
                        ns = slice(nch * NBLK, (nch + 1) * NBLK)
                        qp = psum.tile([128, NBLK], F32, tag="acc", name="acc", bufs=3)
                        for ci in range(2):
                            nc.tensor.matmul(
                                qp[:], wm[ci][:, co * 128:(co + 1) * 128],
                                x1r[ci][:, ns], start=(ci == 0), stop=(ci == 1))
                        nc.scalar.activation(Qt[co][:, ns], qp[:], AF.Identity,
                                             bias=vec[co][:, 0:1])
            with nc.named_scope("blk0"):
                sacc0 = work.tile([128, NBLK], F32R, tag="sacc", name="sacc",
                                  bufs=2)
                emit_block(0, None, sacc0)
            with nc.named_scope("gate"):
                # gate row (x2 columns pre-permuted: query pixels = 0..NH)
                for blk in range(NBLOCKS):
                    ns = slice(blk * NBLK, (blk + 1) * NBLK)
                    gp = psum.tile([1, NBLK], F32, tag="L", name="gp", bufs=2)
                    for ci in range(2):
                        nc.tensor.matmul(gp[:], gw[ci][:, 0:1], x1r[ci][:, ns],
                                         start=(ci == 0), stop=False)
                    for ci in range(2):
                        nc.tensor.matmul(gp[:], gw[ci][:, 1:2], x2r[ci][:, ns],
                                         start=False, stop=(ci == 1))
                    nc.scalar.activation(grow[:, ns], gp[:], AF.Sigmoid,
                                         bias=gb[:])

        prev_fp = None
        prev_sacc = sacc0
        prev_invs = None
        prev = 0
        for blk in range(1, NBLOCKS):
            with nc.named_scope(f"blk{blk}"):
                prev_invs = s_finalize(prev, prev_sacc)
                prev_fp = [psum.tile([128, NBLK], F32, tag="acc", name="acc",
                                     bufs=3) for _ in range(2)]
                sacc = work.tile([128, NBLK], F32R, tag="sacc", name="sacc",
                                 bufs=2)
                emit_block(blk, prev_fp, sacc)
            post_block(prev, prev_fp, prev_invs)
            prev = blk
            prev_sacc = sacc
        with nc.named_scope("tail"):
            prev_invs = s_finalize(prev, prev_sacc)
            prev_fp = [psum.tile([128, NBLK], F32, tag="acc", name="acc", bufs=3)
                       for _ in range(2)]
            for mt in range(MT):
                fusion_mms(prev_fp, mt)
        post_block(prev, prev_fp, prev_invs)
    nc.compile()
    return nc


_NC = None


def _get_nc():
    global _NC
    if _NC is None:
        _NC = build()
    return _NC


def kernel(**inputs):
    x1 = np.ascontiguousarray(np.asarray(inputs["x1"], dtype=np.float32)).reshape(B, C, N)
    x2 = np.ascontiguousarray(np.asarray(inputs["x2"], dtype=np.float32)).reshape(B, C, N)
    q_w = np.asarray(inputs["q_w"], np.float64)
    k_w = np.asarray(inputs["k_w"], np.float64)
    v_w = np.asarray(inputs["v_w"], np.float64)
    p_w = np.asarray(inputs["proj_w"], np.float64)
    q_b = np.asarray(inputs["q_b"], np.float64)
    v_b = np.asarray(inputs["v_b"], np.float64)
    p_b = np.asarray(inputs["proj_b"], np.float64)
    gamma = np.asarray(inputs["bn_gamma"], np.float64)
    beta = np.asarray(inputs["bn_beta"], np.float64)
    mean = np.asarray(inputs["bn_mean"], np.float64)
    var = np.asarray(inputs["bn_var"], np.float64)
    gate_w = np.asarray(inputs["gate_w"], np.float32)
    gate_b = np.asarray(inputs["gate_b"], np.float32)

    # folded weights: Q' = (k_w^T q_w) x1 + k_w^T q_b ;  M1 = (proj_w v_w) Z
    wqkT = (q_w.T @ k_w).astype(np.float32)          # lhsT for Q' projection
    p2T = (v_w.T @ p_w.T).astype(np.float32)         # lhsT for proj stage
    wmat = np.ascontiguousarray(np.concatenate([wqkT, p2T], axis=1))
    gw = np.ascontiguousarray(
        np.stack([gate_w[0, :C], gate_w[0, C:]], axis=1).astype(np.float32))
    G = gamma / np.sqrt(var + EPS)
    Bc = beta + (p_b + p_w @ v_b - mean) * G
    qpb = k_w.T @ q_b
    vecs = np.ascontiguousarray(
        np.stack([qpb, G, Bc, np.zeros(C)], axis=1).astype(np.float32))
    gb = gate_b.reshape(1, 1)

    in_maps = []
    for core in range(NCORES):
        b, half = divmod(core, 2)
        hq = slice(half * NH, (half + 1) * NH)
        ho = slice((1 - half) * NH, (2 - half) * NH)
        x1q = np.ascontiguousarray(x1[b][:, hq])
        x2p = np.ascontiguousarray(np.concatenate([x2[b][:, hq], x2[b][:, ho]],
                                                  axis=1))
        # x2 pretransposed into the fusion lhsT SBUF layout:
        # x2t[p, mt*C + c] = x2p[c, mt*128 + p]
        x2t = np.ascontiguousarray(
            x2p.reshape(C, MT, 128).transpose(2, 1, 0).reshape(128, MT * C))
        in_maps.append({
            "x1r": x1q, "x2r": x2p, "x2t": x2t,
            "wmat": wmat, "gw": gw, "vecs": vecs, "gateb": gb,
        })

    nc = _get_nc()
    res = run_bass_kernel_spmd(nc, in_maps, core_ids=list(range(NCORES)))
    out = np.empty((B, C, N), np.float32)
    for core in range(NCORES):
        b, half = divmod(core, 2)
        out[b, :, half * NH:(half + 1) * NH] = res.results[core]["out"]
    return out.reshape(B, C, H, W)



# revision 8
# speedup vs baseline: 1.4852x; 1.4852x over previous
"""CrossAttentionFusion Trainium2 kernel — fp8 DoubleRow edition.

Full inputs -> shard (batch x query-half) over 8 NeuronCores -> full output.

Per core (batch b = core//2, query half h = core%2, NH=2048 queries):
  Algebraic folding (host precompute):
    L[m,n] = K^T Q = x2^T (k_w^T q_w) x1 =: x2^T Q'   (K never materialized)
    F_att  = v_w (x2 A_norm) + v_b  ->  M1 = (proj_w v_w) Z,  Z = x2 E
    Bc fold: M1_aug = M1 + (Bc/G) (x) S  via an extra K=1 matmul with the
    softmax-sum row S, so  out = x1 + relu(M1_aug * G[c] * (invs*gate)[n]).
  Precision: logits, fusion, and softmax-sum matmuls run in fp8e4m3 with
  MatmulPerfMode.DoubleRow (0.5 cycles/row, K=256 per instruction; 4x the
  fp32r rate).  exp is shifted by -1.5 (softmax-invariant) so E fits fp8
  range (max ~240).  Q' projection / proj / gate-x1 stay fp32r.  Emulated
  end-to-end rel err ~2.4e-3 vs the 2e-2 gate.
  Per 512-query block j: logits(j) via DoubleRow, exp on ACT (the
  bottleneck engine: does exp only), fusion(j-1) + softmax-sum matmuls
  interleaved on the PE, post(j-1) = proj + fused scale/relu/residual on
  DVE+Pool.
"""
from contextlib import ExitStack

import numpy as np
import ml_dtypes

import concourse.bass as bass
import concourse.mybir as mybir
import concourse.tile as tile
from concourse import bacc
from concourse.bass_utils import run_bass_kernel_spmd

F32 = mybir.dt.float32
F32R = mybir.dt.float32r
F8 = mybir.dt.float8e4
AF = mybir.ActivationFunctionType
OP = mybir.AluOpType
DR = mybir.MatmulPerfMode.DoubleRow

B, C, H, W = 4, 256, 64, 64
N = H * W            # 4096
NCORES = 8
NH = N // 2          # 2048 queries per core
NBLK = 512           # query block
NBLOCKS = NH // NBLK
MT = N // 128        # 32 m-tiles
MT2 = MT // 2        # 16 m-tile pairs
EPS = 1e-5
SCALE = float(C) ** -0.5
ESHIFT = -1.5        # exp(L*SCALE + ESHIFT): keeps E < ~70 (fp8e4 max 240)


def build():
    nc = bacc.Bacc("TRN2", target_bir_lowering=False, debug=False,
                   num_devices=NCORES)
    x1r_d = nc.dram_tensor("x1r", [C, NH], F32R, kind="ExternalInput")
    x2r_d = nc.dram_tensor("x2r8", [128, 2 * N], F8, kind="ExternalInput")
    x2t_d = nc.dram_tensor("x2t8", [128, MT * C], F8, kind="ExternalInput")
    wm_d = nc.dram_tensor("wmat", [C, 2 * C], F32R, kind="ExternalInput")
    gw_d = nc.dram_tensor("gw1", [C, 32], F32R, kind="ExternalInput")
    gw8_d = nc.dram_tensor("gw8", [128, 2 * 32], F8, kind="ExternalInput")
    vec_d = nc.dram_tensor("vecs", [C, 2], F32, kind="ExternalInput")
    bg_d = nc.dram_tensor("bgrow", [1, C], F32R, kind="ExternalInput")
    gb_d = nc.dram_tensor("gateb", [1, 1], F32, kind="ExternalInput")
    out_d = nc.dram_tensor("out", [C, NH], F32, kind="ExternalOutput")

    with tile.TileContext(nc) as tc, ExitStack() as ctx:
        pers = ctx.enter_context(tc.tile_pool(name="pers", bufs=1))
        work = ctx.enter_context(tc.tile_pool(name="work", bufs=2))
        psum = ctx.enter_context(tc.tile_pool(name="psum", bufs=1, space="PSUM"))

        # ---- persistent tiles ----
        wm = [pers.tile([128, 2 * C], F32R, tag=f"wm{ci}", name=f"wm{ci}") for ci in range(2)]
        gw = [pers.tile([128, 32], F32R, tag=f"gw{ci}", name=f"gw{ci}") for ci in range(2)]
        gw8 = pers.tile([128, 2, 32], F8, tag="gw8", name="gw8")
        vec = [pers.tile([128, 2], F32, tag=f"vec{ci}", name=f"vec{ci}") for ci in range(2)]
        bgrow = pers.tile([1, C], F32R, tag="bgrow", name="bgrow")
        gb = pers.tile([1, 1], F32, tag="gb", name="gb")
        x1r = [pers.tile([128, NH], F32R, tag=f"x1r{ci}", name=f"x1r{ci}") for ci in range(2)]
        x2r = pers.tile([128, 2, N], F8, tag="x2r", name="x2r")
        x2t = pers.tile([128, MT, C], F8, tag="x2t", name="x2t")
        Qt = pers.tile([128, 2, NH], F8, tag="Qt", name="Qt")
        grow = pers.tile([1, NH], F32R, tag="grow", name="grow")
        E = pers.tile([128, MT, NBLK], F8, tag="E", name="E")
        ones_f = pers.tile([1, 128], F32, tag="ones_f", name="ones_f")
        ones_k1 = pers.tile([1, 128], F32R, tag="ones_k1", name="ones_k1")
        ones8 = pers.tile([128, 2, 32], F8, tag="ones8", name="ones8")
        eshift = pers.tile([128, 1], F32, tag="eshift", name="eshift")

        def emit_block(blk, prev_fp, s_ps):
            ns = slice(blk * NBLK, (blk + 1) * NBLK)
            for mt2 in range(MT2):
                lp = psum.tile([128, 2 * NBLK], F32, tag="L", name="L", bufs=2)
                for sub in range(2):
                    mt = 2 * mt2 + sub
                    nc.tensor.matmul(
                        lp[:, sub * NBLK:(sub + 1) * NBLK],
                        x2r[:, :, mt * 128:(mt + 1) * 128],
                        Qt[:, :, ns], perf_mode=DR)
                if prev_fp is not None:
                    for co in range(2):
                        nc.tensor.matmul(
                            prev_fp[co][:],
                            x2t[:, 2 * mt2:2 * mt2 + 2, co * 128:(co + 1) * 128],
                            E[:, 2 * mt2:2 * mt2 + 2, :], perf_mode=DR,
                            start=(mt2 == 0), stop=(mt2 == MT2 - 1))
                if mt2 >= 1:
                    nc.tensor.matmul(
                        s_ps[:], ones8[:], E[:, 2 * mt2 - 2:2 * mt2, :],
                        perf_mode=DR, start=(mt2 == 1), stop=False)
                nc.scalar.activation(
                    E[:, 2 * mt2:2 * mt2 + 2, :], lp[:], AF.Exp,
                    scale=SCALE, bias=eshift[:, 0:1])
            nc.tensor.matmul(s_ps[:], ones8[:], E[:, MT - 2:MT, :],
                             perf_mode=DR, start=False, stop=True)

        def s_finalize(j, s_ps):
            with nc.named_scope(f"sfin{j}"):
                jns = slice(j * NBLK, (j + 1) * NBLK)
                invs = work.tile([1, NBLK], F32, tag="invs", name="invs")
                nc.vector.reciprocal_approx_fast(invs[:], s_ps[0:1, :])
                srow = work.tile([1, NBLK], F32R, tag="srow", name="srow")
                nc.vector.tensor_copy(srow[:], s_ps[0:1, :])
                gi = work.tile([1, NBLK], F32R, tag="gi", name="gi")
                nc.vector.tensor_mul(gi[:], invs[:], grow[:, jns])
            return gi, srow

        def post_block(j, fp, gi, srow):
            jns = slice(j * NBLK, (j + 1) * NBLK)
            with nc.named_scope(f"post{j}"):
                Fs = [work.tile([128, NBLK], F32R, tag=f"Fs{co}", name=f"Fs{co}",
                                bufs=1) for co in range(2)]
                for co in range(2):
                    nc.vector.tensor_copy(Fs[co][:], fp[co][:])
                bc = psum.tile([128, NBLK], F32, tag="acc", name="acc", bufs=3)
                nc.tensor.matmul(bc[:], ones_k1[:], gi[:])
                gi_b = work.tile([128, NBLK], F32, tag="gi_b", name="gi_b",
                                 bufs=1)
                nc.vector.tensor_copy(gi_b[:], bc[:])
                for co in range(2):
                    cs = slice(co * 128, (co + 1) * 128)
                    mp = psum.tile([128, NBLK], F32, tag="acc", name="acc", bufs=3)
                    for ci in range(2):
                        nc.tensor.matmul(
                            mp[:], wm[ci][:, C + co * 128: C + (co + 1) * 128],
                            Fs[ci][:], start=(ci == 0), stop=False)
                    nc.tensor.matmul(mp[:], bgrow[:, cs], srow[:],
                                     start=False, stop=True)
                    t1 = work.tile([128, NBLK], F32, tag="t1", name="t1")
                    nc.vector.scalar_tensor_tensor(
                        t1[:], mp[:], 0.0, gi_b[:],
                        op0=OP.max, op1=OP.mult)
                    ot = work.tile([128, NBLK], F32, tag="ot", name="ot")
                    nc.gpsimd.tensor_add(ot[:], t1[:],
                                         x1r[co][:, jns].bitcast(F32))
                    nc.sync.dma_start(out_d[cs, jns], ot[:])

        with nc.named_scope("pre"):
            # small/metadata first, then x1+wm (Q' deps), x2r (logits),
            # x2t (fusion, needed latest)
            nc.sync.dma_start(wm[0][:], wm_d[0:128, :])
            nc.gpsimd.dma_start(wm[1][:], wm_d[128:256, :])
            for ci in range(2):
                cs = slice(ci * 128, (ci + 1) * 128)
                nc.sync.dma_start(gw[ci][:], gw_d[cs, :])
                nc.sync.dma_start(vec[ci][:], vec_d[cs, :])
            nc.sync.dma_start(gw8[:], gw8_d[:])
            nc.sync.dma_start(bgrow[:], bg_d[:])
            nc.sync.dma_start(gb[:], gb_d[:])
            nc.vector.memset(ones_f[:], 1.0)
            nc.vector.tensor_copy(ones_k1[:], ones_f[:])
            nc.vector.memset(ones8[:], 1.0)
            nc.vector.memset(eshift[:], ESHIFT)
            CH = 1024
            for ch in range(NH // CH):
                chs = slice(ch * CH, (ch + 1) * CH)
                nc.sync.dma_start(x1r[0][:, chs], x1r_d[0:128, chs])
                nc.gpsimd.dma_start(x1r[1][:, chs], x1r_d[128:256, chs])
            # x2r: chunks split across both queues
            for ch in range(4):
                chs = slice(ch * 2 * N // 4, (ch + 1) * 2 * N // 4)
                eng = nc.sync if ch % 2 == 0 else nc.gpsimd
                eng.dma_start(
                    x2r[:, ch // 2, (ch % 2) * (N // 2):(ch % 2 + 1) * (N // 2)],
                    x2r_d[:, chs])
            for ch in range(2):
                eng = nc.sync if ch == 0 else nc.gpsimd
                eng.dma_start(x2t[:, ch * MT // 2:(ch + 1) * MT // 2, :],
                              x2t_d[:, ch * MT * C // 2:(ch + 1) * MT * C // 2])

            # Q' projection (fp32r) -> fp8 Qt, bias on DVE
            for co in range(2):
                for nch in range(NH // NBLK):
                    ns = slice(nch * NBLK, (nch + 1) * NBLK)
                    qp = psum.tile([128, NBLK], F32, tag="acc", name="acc", bufs=3)
                    for ci in range(2):
                        nc.tensor.matmul(
                            qp[:], wm[ci][:, co * 128:(co + 1) * 128],
                            x1r[ci][:, ns], start=(ci == 0), stop=(ci == 1))
                    nc.vector.tensor_scalar_add(
                        Qt[:, co:co + 1, ns], qp[:], vec[co][:, 0:1])

        with nc.named_scope("gate"):
            for blk in range(NBLOCKS):
                ns = slice(blk * NBLK, (blk + 1) * NBLK)
                gp = psum.tile([32, NBLK], F32, tag="L", name="gp", bufs=2)
                nc.tensor.matmul(gp[:], gw8[:], x2r[:, :, ns], perf_mode=DR,
                                 start=True, stop=False)
                for ci in range(2):
                    nc.tensor.matmul(gp[:], gw[ci][:], x1r[ci][:, ns],
                                     start=False, stop=(ci == 1))
                nc.scalar.activation(grow[:, ns], gp[0:1, :], AF.Sigmoid, bias=gb[:])

        with nc.named_scope("blk0"):
            s_ps = psum.tile([32, NBLK], F32, tag="s", name="s", bufs=1)
            emit_block(0, None, s_ps)

        prev_fp = None
        prev_s = s_ps
        prev = 0
        for blk in range(1, NBLOCKS):
            with nc.named_scope(f"blk{blk}"):
                gi, srow = s_finalize(prev, prev_s)
                prev_fp = [psum.tile([128, NBLK], F32, tag="acc", name="acc",
                                     bufs=3) for _ in range(2)]
                s_ps = psum.tile([32, NBLK], F32, tag="s", name="s", bufs=1)
                emit_block(blk, prev_fp, s_ps)
            post_block(prev, prev_fp, gi, srow)
            prev = blk
            prev_s = s_ps
        with nc.named_scope("tail"):
            gi, srow = s_finalize(prev, prev_s)
            prev_fp = [psum.tile([128, NBLK], F32, tag="acc", name="acc", bufs=3)
                       for _ in range(2)]
            for mt2 in range(MT2):
                for co in range(2):
                    nc.tensor.matmul(
                        prev_fp[co][:],
                        x2t[:, 2 * mt2:2 * mt2 + 2, co * 128:(co + 1) * 128],
                        E[:, 2 * mt2:2 * mt2 + 2, :], perf_mode=DR,
                        start=(mt2 == 0), stop=(mt2 == MT2 - 1))
        post_block(prev, prev_fp, gi, srow)
    nc.compile()
    return nc


_NC = None


def _get_nc():
    global _NC
    if _NC is None:
        _NC = build()
    return _NC


def kernel(**inputs):
    x1 = np.ascontiguousarray(np.asarray(inputs["x1"], dtype=np.float32)).reshape(B, C, N)
    x2 = np.ascontiguousarray(np.asarray(inputs["x2"], dtype=np.float32)).reshape(B, C, N)
    q_w = np.asarray(inputs["q_w"], np.float64)
    k_w = np.asarray(inputs["k_w"], np.float64)
    v_w = np.asarray(inputs["v_w"], np.float64)
    p_w = np.asarray(inputs["proj_w"], np.float64)
    q_b = np.asarray(inputs["q_b"], np.float64)
    v_b = np.asarray(inputs["v_b"], np.float64)
    p_b = np.asarray(inputs["proj_b"], np.float64)
    gamma = np.asarray(inputs["bn_gamma"], np.float64)
    beta = np.asarray(inputs["bn_beta"], np.float64)
    mean = np.asarray(inputs["bn_mean"], np.float64)
    var = np.asarray(inputs["bn_var"], np.float64)
    gate_w = np.asarray(inputs["gate_w"], np.float32)
    gate_b = np.asarray(inputs["gate_b"], np.float32)

    # folded weights: Q' = (k_w^T q_w) x1 + k_w^T q_b ;  M1 = (proj_w v_w) Z
    wqkT = (q_w.T @ k_w).astype(np.float32)          # lhsT for Q' projection
    G = gamma / np.sqrt(var + EPS)
    p2T = ((v_w.T @ p_w.T) * G[None, :]).astype(np.float32)  # proj lhsT, G folded
    wmat = np.ascontiguousarray(np.concatenate([wqkT, p2T], axis=1))
    gw1 = np.ascontiguousarray(np.repeat(gate_w[0, :C, None], 32, axis=1).astype(np.float32))
    gw8 = np.ascontiguousarray(np.repeat(
        gate_w[0, C:].reshape(2, 128).T[:, :, None], 32,
        axis=2).reshape(128, 64).astype(ml_dtypes.float8_e4m3))
    Bc = beta + (p_b + p_w @ v_b - mean) * G
    qpb = k_w.T @ q_b
    vecs = np.ascontiguousarray(
        np.stack([qpb, G], axis=1).astype(np.float32))
    bgrow = np.ascontiguousarray(Bc[None, :].astype(np.float32))
    gb = gate_b.reshape(1, 1)

    in_maps = []
    for core in range(NCORES):
        b, half = divmod(core, 2)
        hq = slice(half * NH, (half + 1) * NH)
        ho = slice((1 - half) * NH, (2 - half) * NH)
        x1q = np.ascontiguousarray(x1[b][:, hq])
        x2p = np.concatenate([x2[b][:, hq], x2[b][:, ho]], axis=1)
        x2p8 = x2p.astype(ml_dtypes.float8_e4m3)
        # x2r8[p, ci*N + m] = fp8(x2p[ci*128+p, m])
        x2r8 = np.ascontiguousarray(
            x2p8.reshape(2, 128, N).transpose(1, 0, 2).reshape(128, 2 * N))
        # x2t8[p, mt*C + c] = fp8(x2p[c, mt*128 + p])
        x2t8 = np.ascontiguousarray(
            x2p8.reshape(C, MT, 128).transpose(2, 1, 0).reshape(128, MT * C))
        in_maps.append({
            "x1r": x1q, "x2r8": x2r8, "x2t8": x2t8,
            "wmat": wmat, "gw1": gw1, "gw8": gw8, "vecs": vecs,
            "bgrow": bgrow, "gateb": gb,
        })

    nc = _get_nc()
    res = run_bass_kernel_spmd(nc, in_maps, core_ids=list(range(NCORES)))
    out = np.empty((B, C, N), np.float32)
    for core in range(NCORES):
        b, half = divmod(core, 2)
        out[b, :, half * NH:(half + 1) * NH] = res.results[core]["out"]
    return out.reshape(B, C, H, W)


# revision 14
# speedup vs baseline: 1.6199x; 1.0906x over previous
"""CrossAttentionFusion Trainium2 kernel — fp8 DoubleRow edition.

Full inputs -> shard (batch x query-half) over 8 NeuronCores -> full output.

Per core (batch b = core//2, query half h = core%2, NH=2048 queries):
  Algebraic folding (host precompute):
    L[m,n] = K^T Q = x2^T (k_w^T q_w) x1 =: x2^T Q'   (K never materialized)
    F_att  = v_w (x2 A_norm) + v_b  ->  M1 = (proj_w v_w) Z,  Z = x2 E
    G fold: p2T*G and bgrow=Bc so  mp = G*M1 + Bc (x) S  and
    out = x1 + relu(mp) * (gate/S)[n]  with gate = 1/(1+exp(-glogit))
    computed as gi = recip(S * (1 + exp(-glogit))) on DVE — ACT runs Exp
    exclusively (one act-table load, ~t=0).
  Precision: Q', logits, fusion, softmax-sum, and gate matmuls all run in
  fp8e4m3 with MatmulPerfMode.DoubleRow (0.5 cycles/row, K=256 per
  instruction).  exp is shifted by -1.5 (softmax-invariant) so E fits fp8
  range (max 240).  proj/bc matmuls stay fp32r.  Emulated end-to-end rel
  err ~3.5e-3 vs the 2e-2 gate.
  Per 512-query block j: logits(j) DoubleRow -> exp on ACT (bottleneck,
  64 x [128,1024] back-to-back), fusion(j-1) + softmax-sum(j, lag-1)
  interleaved on the PE, post(j-1) = proj + fused relu/scale + residual
  on DVE (Pool only issues pre DMAs — its semaphore handling is slow).
"""
from contextlib import ExitStack

import numpy as np
import ml_dtypes

import concourse.bass as bass
import concourse.mybir as mybir
import concourse.tile as tile
from concourse import bacc
from concourse.bass_utils import run_bass_kernel_spmd

F32 = mybir.dt.float32
F32R = mybir.dt.float32r
F8 = mybir.dt.float8e4
AF = mybir.ActivationFunctionType
OP = mybir.AluOpType
DR = mybir.MatmulPerfMode.DoubleRow

B, C, H, W = 4, 256, 64, 64
N = H * W            # 4096
NCORES = 8
NH = N // 2          # 2048 queries per core
NBLK = 512           # query block
NBLOCKS = NH // NBLK
MT = N // 128        # 32 m-tiles
MT2 = MT // 2        # 16 m-tile pairs
EPS = 1e-5
SCALE = float(C) ** -0.5
ASC = 1.0          # x2 fp8 prescale (unscaled: proven regime)
BSC = 16.0         # Qt = BSC*Q' (wqk8 prescaled by BSC)
GSC = 16.0         # gate logits prescale so gate exp shares the 0.0625 table
ESHIFT = -1.5        # exp(L*SCALE + ESHIFT): keeps E < ~70 (fp8e4 max 240)


def build():
    nc = bacc.Bacc("TRN2", target_bir_lowering=False, debug=False,
                   num_devices=NCORES)
    x1r_d = nc.dram_tensor("x1r", [C, NH], F32R, kind="ExternalInput")
    x18_d = nc.dram_tensor("x18", [128, 2 * NH], F8, kind="ExternalInput")
    x2r_d = nc.dram_tensor("x2r8", [128, 2 * N], F8, kind="ExternalInput")
    x2t_d = nc.dram_tensor("x2t8", [128, MT * C], F8, kind="ExternalInput")
    wqk_d = nc.dram_tensor("wqk8", [128, 2 * C], F8, kind="ExternalInput")
    wm_d = nc.dram_tensor("wmat", [C, C], F32R, kind="ExternalInput")
    g1w_d = nc.dram_tensor("g1w8", [128, 2 * 32], F8, kind="ExternalInput")
    gw8_d = nc.dram_tensor("gw8", [128, 2 * 32], F8, kind="ExternalInput")
    vec_d = nc.dram_tensor("vecs", [C, 2], F32, kind="ExternalInput")
    bg_d = nc.dram_tensor("bgrow", [32, C], F32R, kind="ExternalInput")
    gb_d = nc.dram_tensor("gateb", [1, 1], F32, kind="ExternalInput")
    out_d = nc.dram_tensor("out", [C, NH], F32, kind="ExternalOutput")

    with tile.TileContext(nc) as tc, ExitStack() as ctx:
        pers = ctx.enter_context(tc.tile_pool(name="pers", bufs=1))
        work = ctx.enter_context(tc.tile_pool(name="work", bufs=2))
        psum = ctx.enter_context(tc.tile_pool(name="psum", bufs=1, space="PSUM"))

        # ---- persistent tiles ----
        wm = [pers.tile([128, C], F32R, tag=f"wm{ci}", name=f"wm{ci}") for ci in range(2)]
        wqk8 = pers.tile([128, 2, C], F8, tag="wqk8", name="wqk8")
        g1w8 = pers.tile([128, 2, 32], F8, tag="g1w8", name="g1w8")
        gw8 = pers.tile([128, 2, 32], F8, tag="gw8", name="gw8")
        vec = [pers.tile([128, 2], F32, tag=f"vec{ci}", name=f"vec{ci}") for ci in range(2)]
        bgrow = pers.tile([32, C], F32R, tag="bgrow", name="bgrow")
        gbn = pers.tile([1, 1], F32, tag="gbn", name="gbn")
        x1r = [pers.tile([128, NH], F32R, tag=f"x1r{ci}", name=f"x1r{ci}") for ci in range(2)]
        x18 = pers.tile([128, 2, NH], F8, tag="x18", name="x18")
        x2r = pers.tile([128, 2, N], F8, tag="x2r", name="x2r")
        x2t = pers.tile([128, MT, C], F8, tag="x2t", name="x2t")
        Qt = pers.tile([128, 2, NH], F8, tag="Qt", name="Qt")
        growE = pers.tile([1, NH], F32, tag="growE", name="growE")
        E = pers.tile([128, MT, NBLK], F8, tag="E", name="E")
        ones_f = pers.tile([1, 128], F32, tag="ones_f", name="ones_f")
        ones_k1 = pers.tile([1, 128], F32R, tag="ones_k1", name="ones_k1")
        ones8 = pers.tile([128, 2, 32], F8, tag="ones8", name="ones8")
        eshift = pers.tile([128, 1], F32, tag="eshift", name="eshift")

        def gate_mms(jg):
            ns = slice(jg * NBLK, (jg + 1) * NBLK)
            gp = psum.tile([32, NBLK], F32, tag="acc", name="gp", bufs=3)
            nc.tensor.matmul(gp[:], g1w8[:], x18[:, :, ns], perf_mode=DR,
                             start=True, stop=False)
            nc.tensor.matmul(gp[:], gw8[:], x2r[:, :, ns], perf_mode=DR,
                             start=False, stop=True)
            nc.scalar.activation(growE[:, ns], gp[0:1, :], AF.Exp,
                                 scale=SCALE, bias=gbn[:])

        def emit_block(blk, prev_fp, s_ps, gate=False):
            ns = slice(blk * NBLK, (blk + 1) * NBLK)
            for mt2 in range(MT2):
                lp = psum.tile([128, 2 * NBLK], F32, tag="L", name="L", bufs=2)
                for sub in range(2):
                    mt = 2 * mt2 + sub
                    nc.tensor.matmul(
                        lp[:, sub * NBLK:(sub + 1) * NBLK],
                        x2r[:, :, mt * 128:(mt + 1) * 128],
                        Qt[:, :, ns], perf_mode=DR)
                if prev_fp is not None:
                    for co in range(2):
                        nc.tensor.matmul(
                            prev_fp[co][:],
                            x2t[:, 2 * mt2:2 * mt2 + 2, co * 128:(co + 1) * 128],
                            E[:, 2 * mt2:2 * mt2 + 2, :], perf_mode=DR,
                            start=(mt2 == 0), stop=(mt2 == MT2 - 1))
                if mt2 >= 1:
                    nc.tensor.matmul(
                        s_ps[:], ones8[:], E[:, 2 * mt2 - 2:2 * mt2, :],
                        perf_mode=DR, start=(mt2 == 1), stop=False)
                if gate and mt2 in (2, 5, 8, 11):
                    gate_mms((mt2 - 2) // 3)
                nc.scalar.activation(
                    E[:, 2 * mt2:2 * mt2 + 2, :], lp[:], AF.Exp,
                    scale=SCALE, bias=eshift[:, 0:1])
            nc.tensor.matmul(s_ps[:], ones8[:], E[:, MT - 2:MT, :],
                             perf_mode=DR, start=False, stop=True)

        def s_finalize(j, s_ps):
            # gi = gate/S = 1/(S*(1+exp(-glogit)));  srow = S (for Bc fold)
            with nc.named_scope(f"sfin{j}"):
                jns = slice(j * NBLK, (j + 1) * NBLK)
                tg = work.tile([1, NBLK], F32, tag="tg", name="tg")
                nc.vector.tensor_scalar_add(tg[:], growE[:, jns], 1.0)
                wv = work.tile([1, NBLK], F32, tag="wv", name="wv")
                nc.vector.tensor_mul(wv[:], tg[:], s_ps[0:1, :])
                gif = work.tile([1, NBLK], F32, tag="gif", name="gif")
                nc.vector.reciprocal_approx_fast(gif[:], wv[:])
                gi = work.tile([1, NBLK], F32R, tag="gi", name="gi")
                nc.vector.tensor_copy(gi[:], gif[:])
                srow = work.tile([32, NBLK], F32R, tag="srow", name="srow")
                nc.vector.tensor_copy(srow[:], s_ps[:, :])
            return gi, srow

        def post_block(j, fp, gi, srow):
            jns = slice(j * NBLK, (j + 1) * NBLK)
            with nc.named_scope(f"post{j}"):
                Fs = [work.tile([128, NBLK], F32R, tag=f"Fs{co}", name=f"Fs{co}",
                                bufs=1) for co in range(2)]
                for co in range(2):
                    nc.vector.tensor_copy(Fs[co][:], fp[co][:])
                bc = psum.tile([128, NBLK], F32, tag="acc", name="acc", bufs=3)
                nc.tensor.matmul(bc[:], ones_k1[:], gi[:])
                gi_b = work.tile([128, NBLK], F32, tag="gi_b", name="gi_b",
                                 bufs=1)
                nc.vector.tensor_copy(gi_b[:], bc[:])
                for co in range(2):
                    cs = slice(co * 128, (co + 1) * 128)
                    mp = psum.tile([128, NBLK], F32, tag="acc", name="acc", bufs=3)
                    for ci in range(2):
                        nc.tensor.matmul(
                            mp[:], wm[ci][:, co * 128:(co + 1) * 128],
                            Fs[ci][:], start=(ci == 0), stop=False)
                    nc.tensor.matmul(mp[:], bgrow[:, cs], srow[:],
                                     start=False, stop=True)
                    t1 = work.tile([128, NBLK], F32, tag="t1", name="t1")
                    nc.vector.scalar_tensor_tensor(
                        t1[:], mp[:], 0.0, gi_b[:],
                        op0=OP.max, op1=OP.mult)
                    ot = work.tile([128, NBLK], F32, tag="ot", name="ot")
                    nc.vector.tensor_add(ot[:], t1[:],
                                         x1r[co][:, jns].bitcast(F32))
                    nc.sync.dma_start(out_d[cs, jns], ot[:])

        with nc.named_scope("pre"):
            # critical path to first exp: wqk8+x18 -> Q'(nch0) -> logits(m0)
            nc.sync.dma_start(wqk8[:], wqk_d[:])
            nc.sync.dma_start(x18[:], x18_d[:])
            for ci in range(2):
                nc.sync.dma_start(vec[ci][:], vec_d[ci * 128:(ci + 1) * 128, :])
            nc.gpsimd.dma_start(g1w8[:], g1w_d[:])
            nc.gpsimd.dma_start(gw8[:], gw8_d[:])
            nc.sync.dma_start(bgrow[:], bg_d[:])
            nc.sync.dma_start(gbn[:], gb_d[:])
            nc.vector.memset(ones_f[:], 1.0)
            nc.vector.tensor_copy(ones_k1[:], ones_f[:])
            nc.vector.memset(ones8[:], 1.0)
            nc.vector.memset(eshift[:], ESHIFT)
            # fp8 partial-tile DMAs MUST use gpsimd (swdge): the sync
            # queue's hw descriptor path corrupts strided fp8 (clears low
            # nibbles).  Full-tile flat fp8 (x18, wqk8) is fine on sync.
            # x2r m-major so logits stream against arrival.
            MC = 2048
            for mr in range(N // MC):
                for ci in range(2):
                    nc.gpsimd.dma_start(
                        x2r[:, ci, mr * MC:(mr + 1) * MC],
                        x2r_d[:, ci * N + mr * MC: ci * N + (mr + 1) * MC])
            for ch in range(2):
                nc.gpsimd.dma_start(
                    x2t[:, ch * MT // 2:(ch + 1) * MT // 2, :],
                    x2t_d[:, ch * MT * C // 2:(ch + 1) * MT * C // 2])
            nc.sync.dma_start(wm[0][:], wm_d[0:128, :])
            nc.sync.dma_start(wm[1][:], wm_d[128:256, :])
            for ch in range(2):
                chs = slice(ch * NH // 2, (ch + 1) * NH // 2)
                nc.sync.dma_start(x1r[0][:, chs], x1r_d[0:128, chs])
                nc.sync.dma_start(x1r[1][:, chs], x1r_d[128:256, chs])

            # Q' projection: fp8 DoubleRow + DVE bias -> fp8 Qt
            for nch in range(NH // NBLK):
                for co in range(2):
                    ns = slice(nch * NBLK, (nch + 1) * NBLK)
                    qp = psum.tile([128, NBLK], F32, tag="acc", name="acc", bufs=3)
                    nc.tensor.matmul(qp[:], wqk8[:, :, co * 128:(co + 1) * 128],
                                     x18[:, :, ns], perf_mode=DR)
                    nc.vector.tensor_scalar(
                        Qt[:, co:co + 1, ns], qp[:], vec[co][:, 0:1],
                        1.0 / BSC, op0=OP.add, op1=OP.mult)

        with nc.named_scope("blk0"):
            s_ps = psum.tile([32, NBLK], F32, tag="s", name="s", bufs=1)
            emit_block(0, None, s_ps, gate=True)

        prev_fp = None
        prev_s = s_ps
        prev = 0
        for blk in range(1, NBLOCKS):
            with nc.named_scope(f"blk{blk}"):
                gi, srow = s_finalize(prev, prev_s)
                prev_fp = [psum.tile([128, NBLK], F32, tag="acc", name="acc",
                                     bufs=3) for _ in range(2)]
                s_ps = psum.tile([32, NBLK], F32, tag="s", name="s", bufs=1)
                emit_block(blk, prev_fp, s_ps)
            post_block(prev, prev_fp, gi, srow)
            prev = blk
            prev_s = s_ps
        with nc.named_scope("tail"):
            gi, srow = s_finalize(prev, prev_s)
            prev_fp = [psum.tile([128, NBLK], F32, tag="acc", name="acc", bufs=3)
                       for _ in range(2)]
            for mt2 in range(MT2):
                for co in range(2):
                    nc.tensor.matmul(
                        prev_fp[co][:],
                        x2t[:, 2 * mt2:2 * mt2 + 2, co * 128:(co + 1) * 128],
                        E[:, 2 * mt2:2 * mt2 + 2, :], perf_mode=DR,
                        start=(mt2 == 0), stop=(mt2 == MT2 - 1))
        post_block(prev, prev_fp, gi, srow)
    nc.compile()
    return nc


_NC = None


def _get_nc():
    global _NC
    if _NC is None:
        _NC = build()
    return _NC


def kernel(**inputs):
    x1 = np.ascontiguousarray(np.asarray(inputs["x1"], dtype=np.float32)).reshape(B, C, N)
    x2 = np.ascontiguousarray(np.asarray(inputs["x2"], dtype=np.float32)).reshape(B, C, N)
    q_w = np.asarray(inputs["q_w"], np.float64)
    k_w = np.asarray(inputs["k_w"], np.float64)
    v_w = np.asarray(inputs["v_w"], np.float64)
    p_w = np.asarray(inputs["proj_w"], np.float64)
    q_b = np.asarray(inputs["q_b"], np.float64)
    v_b = np.asarray(inputs["v_b"], np.float64)
    p_b = np.asarray(inputs["proj_b"], np.float64)
    gamma = np.asarray(inputs["bn_gamma"], np.float64)
    beta = np.asarray(inputs["bn_beta"], np.float64)
    mean = np.asarray(inputs["bn_mean"], np.float64)
    var = np.asarray(inputs["bn_var"], np.float64)
    gate_w = np.asarray(inputs["gate_w"], np.float32)
    gate_b = np.asarray(inputs["gate_b"], np.float32)

    FP8 = ml_dtypes.float8_e4m3
    # folded weights: Q' = (k_w^T q_w) x1 + k_w^T q_b ;  M1 = (proj_w v_w) Z
    wqkT = (q_w.T @ k_w).astype(np.float32)          # [c_in, c_out] lhsT
    G = gamma / np.sqrt(var + EPS)
    p2T = ((v_w.T @ p_w.T) * G[None, :]).astype(np.float32)  # G folded
    # wqk8[p, ci*C + j] = fp8(wqkT[ci*128+p, j])
    wqk8 = np.ascontiguousarray(
        (BSC * wqkT).astype(FP8).reshape(2, 128, C).transpose(1, 0, 2).reshape(128, 2 * C))
    wmat = np.ascontiguousarray(p2T)
    # negated gate weights (gate computed via exp(-logit))
    g1s = np.clip(-gate_w[0, :C] * GSC, -200, 200)
    g2s = np.clip(-gate_w[0, C:] * GSC, -200, 200)
    g1w8 = np.ascontiguousarray(np.repeat(
        g1s.reshape(2, 128).T[:, :, None], 32,
        axis=2).reshape(128, 64).astype(FP8))
    gw8 = np.ascontiguousarray(np.repeat(
        g2s.reshape(2, 128).T[:, :, None], 32,
        axis=2).reshape(128, 64).astype(FP8))
    Bc = beta + (p_b + p_w @ v_b - mean) * G
    qpb = k_w.T @ q_b
    vecs = np.ascontiguousarray(
        np.stack([BSC * qpb, G], axis=1).astype(np.float32))
    bgrow = np.ascontiguousarray(np.repeat((Bc / 32.0)[None, :], 32, 0).astype(np.float32))
    gb = np.ascontiguousarray((-gate_b).reshape(1, 1).astype(np.float32))

    in_maps = []
    for core in range(NCORES):
        b, half = divmod(core, 2)
        hq = slice(half * NH, (half + 1) * NH)
        ho = slice((1 - half) * NH, (2 - half) * NH)
        x1q = np.ascontiguousarray(x1[b][:, hq])
        x18 = np.ascontiguousarray(
            x1q.astype(FP8).reshape(2, 128, NH).transpose(1, 0, 2).reshape(128, 2 * NH))
        x2p = np.concatenate([x2[b][:, hq], x2[b][:, ho]], axis=1)
        x2p8 = x2p.astype(FP8)
        # x2r8[p, ci*N + m] = fp8(x2p[ci*128+p, m])
        x2r8 = np.ascontiguousarray(
            x2p8.reshape(2, 128, N).transpose(1, 0, 2).reshape(128, 2 * N))
        # x2t8[p, mt*C + c] = fp8(x2p[c, mt*128 + p])
        x2t8 = np.ascontiguousarray(
            x2p8.reshape(C, MT, 128).transpose(2, 1, 0).reshape(128, MT * C))
        in_maps.append({
            "x1r": x1q, "x18": x18, "x2r8": x2r8, "x2t8": x2t8,
            "wqk8": wqk8, "wmat": wmat, "g1w8": g1w8, "gw8": gw8,
            "vecs": vecs, "bgrow": bgrow, "gateb": gb,
        })

    nc = _get_nc()
    res = run_bass_kernel_spmd(nc, in_maps, core_ids=list(range(NCORES)))
    out = np.empty((B, C, N), np.float32)
    for core in range(NCORES):
        b, half = divmod(core, 2)
        out[b, :, half * NH:(half + 1) * NH] = res.results[core]["out"]
    return out.reshape(B, C, H, W)
